# revision 8
# baseline (speedup 1.0000x reference)
"""GQA attention (BagleyAttention) on 8 Trainium2 NeuronCores.

Tensor-parallel over kv-head groups: core c owns kv head c and query heads
[4c, 4c+4). Each core computes its heads' attention and a partial output
projection [S, D]; the host sums the 8 partials.

All matmuls run in fp32r (TF32-like) at full PE rate; operands are rounded
to fp32r on-chip by DVE/ACT ops (walrus requires fp32r matmul inputs to be
produced by a rounding instruction).
"""

import math
import sys

sys.path.insert(0, "/opt/trn_rl_repo")

import numpy as np

# Problem sizes (hardcoded per contract; kernel.py reads no sibling files).
B, S, D = 1, 2048, 4096
H, KV, Dh = 32, 8, 128
G = H // KV            # query heads per kv head (= per core)
EH = G * Dh            # per-core q projection width (512)
N_CORES = 8

SB = 512               # s-block width for projections / q-block width
N_SB = S // SB         # 4
N_DC = D // 128        # 32 d-chunks
N_ST = S // 128        # 16 s-tiles of 128
N_NB = D // SB         # 8 output d-blocks

_cache = {}


def _build():
    import concourse.bass as bass
    import concourse.mybir as mybir
    import concourse.tile as tile
    from concourse import bacc
    from concourse.masks import make_identity

    dt = mybir.dt
    f32, f32r = dt.float32, dt.float32r
    AF = mybir.ActivationFunctionType

    nc = bacc.Bacc("TRN2", target_bir_lowering=False, debug=False)

    xT = nc.dram_tensor("xT", [D, S], f32, kind="ExternalInput").ap()
    wqT = nc.dram_tensor("wqT", [D, EH], f32, kind="ExternalInput").ap()
    wkT = nc.dram_tensor("wkT", [D, Dh], f32, kind="ExternalInput").ap()
    wvT = nc.dram_tensor("wvT", [D, Dh], f32, kind="ExternalInput").ap()
    woT = nc.dram_tensor("woT", [EH, D], f32, kind="ExternalInput").ap()
    cosT = nc.dram_tensor("cosT", [Dh, S], f32, kind="ExternalInput").ap()
    sinmT = nc.dram_tensor("sinmT", [Dh, S], f32, kind="ExternalInput").ap()
    trim = nc.dram_tensor("trim", [128, 128], f32r, kind="ExternalInput").ap()
    out = nc.dram_tensor("out", [S, D], f32, kind="ExternalOutput").ap()

    with tile.TileContext(nc) as tc, \
         tc.tile_pool(name="persist", bufs=1) as persist:
        # ---- long-lived tensors -------------------------------------------
        # RoPE'd projections, transposed layout [Dh, S], fp32r
        qr = [persist.tile([128, S], f32r, tag=f"qr{h}", name=f"qr{h}")
              for h in range(G)]
        kr = persist.tile([128, S], f32r, tag="kr")
        # V in natural layout: [s-local 128, (s-tile, Dh)]
        vnat = persist.tile([128, N_ST, Dh], f32r, tag="vnat")
        # small constants
        trim_sb = persist.tile([128, 128], f32r, tag="trim")
        ones_r = persist.tile([128, 1], f32r, tag="ones")
        ident = persist.tile([128, 128], f32, tag="ident")

        nc.sync.dma_start(out=trim_sb, in_=trim)  # exact 0/1 values
        ones_f = persist.tile([128, 1], f32, tag="ones_f")
        nc.vector.memset(ones_f, 1.0)
        nc.vector.tensor_copy(out=ones_r, in_=ones_f)
        make_identity(nc, ident)

        # ================= Phase 1: projections + RoPE =====================
        with tc.tile_pool(name="wts", bufs=1) as wts, \
             tc.tile_pool(name="wstage", bufs=2) as wstage, \
             tc.tile_pool(name="trig", bufs=1) as trig, \
             tc.tile_pool(name="xstage", bufs=3) as xstage, \
             tc.tile_pool(name="rope", bufs=2) as rope, \
             tc.tile_pool(name="p1psum", bufs=1, space="PSUM") as p1psum, \
             tc.tile_pool(name="tpsum", bufs=2, space="PSUM") as tpsum:

            # weights: [128(d-local), d-chunk, e] fp32r; per-chunk DMA+round
            # so the first projection matmuls aren't gated on the full 12MB.
            wq_r = wts.tile([128, N_DC, EH], f32r, tag="wq_r")
            wk_r = wts.tile([128, N_DC, Dh], f32r, tag="wk_r")
            wv_r = wts.tile([128, N_DC, Dh], f32r, tag="wv_r")
            wq_re = wqT.rearrange("(c p) e -> p c e", p=128)
            wk_re = wkT.rearrange("(c p) e -> p c e", p=128)
            wv_re = wvT.rearrange("(c p) e -> p c e", p=128)
            for dc in range(N_DC):
                for src, dst, width in ((wq_re, wq_r, EH), (wk_re, wk_r, Dh),
                                        (wv_re, wv_r, Dh)):
                    wf = wstage.tile([128, EH], f32, tag="wf", name="wf")
                    nc.sync.dma_start(out=wf[:, :width], in_=src[:, dc, :])
                    nc.vector.tensor_copy(out=dst[:, dc, :], in_=wf[:, :width])

            cos_sb = trig.tile([128, S], f32, tag="cos")
            sinm_sb = trig.tile([128, S], f32, tag="sinm")
            nc.sync.dma_start(out=cos_sb, in_=cosT)
            nc.sync.dma_start(out=sinm_sb, in_=sinmT)

            for sb in range(N_SB):
                ss = slice(sb * SB, (sb + 1) * SB)
                # 6 PSUM accumulators: 4 q heads + k + v, all [e=128, s=512]
                acc = [p1psum.tile([128, SB], f32, tag=f"acc{i}", name=f"acc{i}")
                       for i in range(6)]
                for dc in range(N_DC):
                    xf = xstage.tile([128, SB], f32, tag="xf")
                    nc.sync.dma_start(out=xf, in_=xT[dc * 128:(dc + 1) * 128, ss])
                    xr = xstage.tile([128, SB], f32r, tag="xr")
                    nc.vector.tensor_copy(out=xr, in_=xf)
                    st_flags = dict(start=(dc == 0), stop=(dc == N_DC - 1))
                    for h in range(G):
                        nc.tensor.matmul(
                            acc[h][:], wq_r[:, dc, h * 128:(h + 1) * 128], xr[:],
                            **st_flags)
                    nc.tensor.matmul(acc[4][:], wk_r[:, dc, :], xr[:], **st_flags)
                    nc.tensor.matmul(acc[5][:], wv_r[:, dc, :], xr[:], **st_flags)

                # RoPE for 4 q heads + k -> fp32r into persistent tiles
                for i in range(5):
                    dst = qr[i][:, ss] if i < G else kr[:, ss]
                    t_cos = rope.tile([128, SB], f32, tag="t_cos")
                    nc.vector.tensor_mul(t_cos, acc[i][:], cos_sb[:, ss])
                    t_plain = rope.tile([128, SB], f32, tag="t_plain")
                    nc.scalar.copy(out=t_plain, in_=acc[i][:])
                    t_swap = rope.tile([128, SB], f32, tag="t_swap")
                    nc.sync.dma_start(out=t_swap[0:64, :], in_=t_plain[64:128, :])
                    nc.sync.dma_start(out=t_swap[64:128, :], in_=t_plain[0:64, :])
                    nc.vector.tensor_mul(t_swap, t_swap, sinm_sb[:, ss])
                    nc.vector.tensor_add(dst, t_cos, t_swap)

                # V: copy PSUM->SBUF, PE-transpose 128x128 blocks to natural
                vt_sb = rope.tile([128, SB], f32, tag="vt_sb")
                nc.scalar.copy(out=vt_sb, in_=acc[5][:])
                for j in range(SB // 128):
                    tp = tpsum.tile([128, 128], f32, tag="tp")
                    nc.tensor.transpose(tp[:], vt_sb[:, j * 128:(j + 1) * 128],
                                        ident[:])
                    nc.scalar.copy(out=vnat[:, sb * 4 + j, :], in_=tp[:])

        # ================= Phase 2: attention ==============================
        inv_sqrt_dh = 1.0 / math.sqrt(Dh)
        with tc.tile_pool(name="wo_pool", bufs=1) as wo_pool, \
             tc.tile_pool(name="attn_pool", bufs=1) as attn_pool, \
             tc.tile_pool(name="wo_stage", bufs=2) as wo_stage:

            # unnormalized attn^T per head [Dh, S], fp32r
            attn = [attn_pool.tile([128, S], f32r, tag=f"attn{h}",
                                   name=f"attn{h}") for h in range(G)]
            # load + round Wo during attention (overlaps with PE work)
            wo_r = wo_pool.tile([128, G, D], f32r, tag="wo_r")
            woT_re = woT.rearrange("(h p) d -> p h d", p=128)
            WOC = 1024
            for h in range(G):
                for wc in range(D // WOC):
                    wo_f = wo_stage.tile([128, WOC], f32, tag="wo_f", name="wo_f")
                    nc.sync.dma_start(out=wo_f,
                                      in_=woT_re[:, h, wc * WOC:(wc + 1) * WOC])
                    nc.vector.tensor_copy(
                        out=wo_r[:, h, wc * WOC:(wc + 1) * WOC], in_=wo_f)

            with tc.tile_pool(name="expp", bufs=4) as expp, \
                 tc.tile_pool(name="zpool", bufs=2) as zpool, \
                 tc.tile_pool(name="scps", bufs=2, space="PSUM") as scps, \
                 tc.tile_pool(name="sumps", bufs=2, space="PSUM") as sumps, \
                 tc.tile_pool(name="pvps", bufs=2, space="PSUM") as pvps:
                for t in range(N_SB):
                    qs = slice(t * SB, (t + 1) * SB)
                    n_chunks = 4 * (t + 1)
                    for h in range(G):
                        pv_ps = pvps.tile([128, SB], f32, tag="pv")
                        sum_ps = sumps.tile([1, SB], f32, tag="sum")
                        for c in range(n_chunks):
                            sc = scps.tile([128, SB], f32, tag="sc")
                            nc.tensor.matmul(
                                sc[:], kr[:, c * 128:(c + 1) * 128],
                                qr[h][:, qs], start=True, stop=True)
                            e = expp.tile([128, SB], f32r, tag="e")
                            nc.scalar.activation(e[:], sc[:], AF.Exp,
                                                 scale=inv_sqrt_dh)
                            j = c - 4 * t
                            if j >= 0:  # chunk contains the causal diagonal
                                nc.vector.tensor_mul(
                                    e[:, j * 128:(j + 1) * 128],
                                    e[:, j * 128:(j + 1) * 128], trim_sb[:])
                                if j > 0:
                                    nc.vector.memset(e[:, 0:j * 128].bitcast(f32), 0.0)
                            mmf = dict(start=(c == 0), stop=(c == n_chunks - 1))
                            nc.tensor.matmul(sum_ps[:], ones_r[:], e[:], **mmf)
                            nc.tensor.matmul(pv_ps[:], vnat[:, c, :], e[:], **mmf)

                        z_sb = zpool.tile([1, SB], f32, tag="z")
                        nc.vector.tensor_copy(out=z_sb, in_=sum_ps[:])
                        rinv = zpool.tile([1, SB], f32, tag="rinv")
                        nc.vector.reciprocal(out=rinv, in_=z_sb)
                        rbc = zpool.tile([128, SB], f32, tag="rbc")
                        nc.gpsimd.partition_broadcast(rbc[:], rinv[:])
                        nc.vector.tensor_mul(attn[h][:, qs], pv_ps[:], rbc[:])

            # ============= Phase 3: output projection ======================
            with tc.tile_pool(name="obuf", bufs=3) as obuf, \
                 tc.tile_pool(name="ops", bufs=3, space="PSUM") as ops:
                for st in range(N_ST):
                    rs = slice(st * 128, (st + 1) * 128)
                    for nb in range(N_NB):
                        cs = slice(nb * SB, (nb + 1) * SB)
                        op = ops.tile([128, SB], f32, tag="op")
                        for h in range(G):
                            nc.tensor.matmul(op[:], attn[h][:, rs],
                                             wo_r[:, h, cs],
                                             start=(h == 0), stop=(h == G - 1))
                        ob = obuf.tile([128, SB], f32, tag="ob")
                        nc.scalar.copy(out=ob, in_=op[:])
                        nc.sync.dma_start(out=out[rs, cs], in_=ob)

    nc.compile()
    return nc


def _prep_inputs(hidden_states, Wq, Wk, Wv, Wo, cos, sin):
    x = np.asarray(hidden_states, dtype=np.float32).reshape(S, D)
    Wq = np.asarray(Wq, dtype=np.float32)
    Wk = np.asarray(Wk, dtype=np.float32)
    Wv = np.asarray(Wv, dtype=np.float32)
    Wo = np.asarray(Wo, dtype=np.float32)
    cos = np.asarray(cos, dtype=np.float32)
    sin = np.asarray(sin, dtype=np.float32)

    xT = np.ascontiguousarray(x.T)
    cosT = np.ascontiguousarray(cos.T)
    sinmT = np.ascontiguousarray(sin.T).copy()
    sinmT[: Dh // 2] *= -1.0
    trimask = np.triu(np.ones((128, 128), dtype=np.float32))  # kpos<=q valid

    in_maps = []
    for c in range(N_CORES):
        in_maps.append({
            "xT": xT,
            "wqT": np.ascontiguousarray(Wq[c * EH:(c + 1) * EH, :].T),
            "wkT": np.ascontiguousarray(Wk[c * Dh:(c + 1) * Dh, :].T),
            "wvT": np.ascontiguousarray(Wv[c * Dh:(c + 1) * Dh, :].T),
            "woT": np.ascontiguousarray(Wo[:, c * EH:(c + 1) * EH].T),
            "cosT": cosT,
            "sinmT": sinmT,
            "trim": trimask,
        })
    return in_maps


def run(trace=False, **inputs):
    """Run on hardware; returns (full_output, exec_time_ns or None)."""
    from concourse.bass_utils import run_bass_kernel_spmd

    if trace:
        _install_ntff_hook()
    if "nc" not in _cache:
        _cache["nc"] = _build()
    nc = _cache["nc"]
    in_maps = _prep_inputs(**inputs)
    res = run_bass_kernel_spmd(nc, in_maps, core_ids=list(range(N_CORES)),
                               trace=trace)
    acc = res.results[0]["out"].astype(np.float32)
    for c in range(1, N_CORES):
        acc += res.results[c]["out"]
    return acc.reshape(B, S, D), res.exec_time_ns


def _install_ntff_hook():
    """Register the axon NTFF profiling hook missing from this image."""
    import types
    try:
        import antenv
        from trn_agent_boot.trn_boot import _ntff_profile_via_ctypes
    except ImportError:
        return
    if "antenv.axon_hooks" in sys.modules:
        return
    mod = types.ModuleType("antenv.axon_hooks")
    mod._hook = _ntff_profile_via_ctypes("/opt/axon/libaxon_pjrt.so")
    mod.get_axon_ntff_profile_hook = lambda: mod._hook
    mod.set_axon_ntff_profile_hook = lambda h: setattr(mod, "_hook", h)
    sys.modules["antenv.axon_hooks"] = mod
    antenv.axon_hooks = mod


def kernel(**inputs):
    out, _ = run(trace=False, **inputs)
    return out


# revision 9
# speedup vs baseline: 1.0277x; 1.0277x over previous
"""GQA attention (BagleyAttention) on 8 Trainium2 NeuronCores.

Tensor-parallel over kv-head groups: core c owns kv head c and query heads
[4c, 4c+4). Each core computes its heads' attention and a partial output
projection [S, D]; the host sums the 8 partials.

All matmuls run in fp32r (TF32-like) at full PE rate; operands are rounded
to fp32r on-chip by DVE/ACT ops (walrus requires fp32r matmul inputs to be
produced by a rounding instruction).
"""

import math
import sys

sys.path.insert(0, "/opt/trn_rl_repo")

import ml_dtypes
import numpy as np

# Problem sizes (hardcoded per contract; kernel.py reads no sibling files).
B, S, D = 1, 2048, 4096
H, KV, Dh = 32, 8, 128
G = H // KV            # query heads per kv head (= per core)
EH = G * Dh            # per-core q projection width (512)
N_CORES = 8

SB = 512               # s-block width for projections / q-block width
N_SB = S // SB         # 4
N_DC = D // 128        # 32 d-chunks
N_ST = S // 128        # 16 s-tiles of 128
N_NB = D // SB         # 8 output d-blocks

_cache = {}


def _build():
    import concourse.bass as bass
    import concourse.mybir as mybir
    import concourse.tile as tile
    from concourse import bacc
    from concourse.masks import make_identity

    dt = mybir.dt
    f32, f32r = dt.float32, dt.float32r
    bf16 = dt.bfloat16
    AF = mybir.ActivationFunctionType

    nc = bacc.Bacc("TRN2", target_bir_lowering=False, debug=False)

    xT = nc.dram_tensor("xT", [D, S], f32, kind="ExternalInput").ap()
    wqT = nc.dram_tensor("wqT", [D, EH], f32, kind="ExternalInput").ap()
    wkT = nc.dram_tensor("wkT", [D, Dh], f32, kind="ExternalInput").ap()
    wvT = nc.dram_tensor("wvT", [D, Dh], f32, kind="ExternalInput").ap()
    woT = nc.dram_tensor("woT", [EH, D], bf16, kind="ExternalInput").ap()
    cosT = nc.dram_tensor("cosT", [Dh, S], f32, kind="ExternalInput").ap()
    sinmT = nc.dram_tensor("sinmT", [Dh, S], f32, kind="ExternalInput").ap()
    trim = nc.dram_tensor("trim", [128, 128], bf16, kind="ExternalInput").ap()
    out = nc.dram_tensor("out", [S, D], f32, kind="ExternalOutput").ap()

    with tile.TileContext(nc) as tc, \
         tc.tile_pool(name="persist", bufs=1) as persist:
        # ---- long-lived tensors -------------------------------------------
        # RoPE'd projections, transposed layout [Dh, S], fp32r
        qr = [persist.tile([128, S], f32r, tag=f"qr{h}", name=f"qr{h}")
              for h in range(G)]
        kr = persist.tile([128, S], f32r, tag="kr")
        # V in natural layout: [s-local 128, (s-tile, Dh)]
        vnat = persist.tile([128, N_ST, Dh], bf16, tag="vnat")
        # small constants
        trim_sb = persist.tile([128, 128], bf16, tag="trim")
        ones_b = persist.tile([128, 1], bf16, tag="ones")
        ident = persist.tile([128, 128], f32, tag="ident")

        nc.sync.dma_start(out=trim_sb, in_=trim)  # exact 0/1 values
        nc.vector.memset(ones_b, 1.0)
        make_identity(nc, ident)

        # ================= Phase 1: projections + RoPE =====================
        with tc.tile_pool(name="wts", bufs=1) as wts, \
             tc.tile_pool(name="wstage", bufs=2) as wstage, \
             tc.tile_pool(name="trig", bufs=1) as trig, \
             tc.tile_pool(name="xstage", bufs=3) as xstage, \
             tc.tile_pool(name="rope", bufs=2) as rope, \
             tc.tile_pool(name="p1psum", bufs=1, space="PSUM") as p1psum, \
             tc.tile_pool(name="tpsum", bufs=2, space="PSUM") as tpsum:

            # weights: [128(d-local), d-chunk, e] fp32r; per-chunk DMA+round
            # so the first projection matmuls aren't gated on the full 12MB.
            wq_r = wts.tile([128, N_DC, EH], f32r, tag="wq_r")
            wk_r = wts.tile([128, N_DC, Dh], f32r, tag="wk_r")
            wv_r = wts.tile([128, N_DC, Dh], f32r, tag="wv_r")
            wq_re = wqT.rearrange("(c p) e -> p c e", p=128)
            wk_re = wkT.rearrange("(c p) e -> p c e", p=128)
            wv_re = wvT.rearrange("(c p) e -> p c e", p=128)
            for dc in range(N_DC):
                for src, dst, width in ((wq_re, wq_r, EH), (wk_re, wk_r, Dh),
                                        (wv_re, wv_r, Dh)):
                    wf = wstage.tile([128, EH], f32, tag="wf", name="wf")
                    nc.sync.dma_start(out=wf[:, :width], in_=src[:, dc, :])
                    nc.vector.tensor_copy(out=dst[:, dc, :], in_=wf[:, :width])

            cos_sb = trig.tile([128, S], f32, tag="cos")
            sinm_sb = trig.tile([128, S], f32, tag="sinm")
            nc.sync.dma_start(out=cos_sb, in_=cosT)
            nc.sync.dma_start(out=sinm_sb, in_=sinmT)

            for sb in range(N_SB):
                ss = slice(sb * SB, (sb + 1) * SB)
                # 6 PSUM accumulators: 4 q heads + k + v, all [e=128, s=512]
                acc = [p1psum.tile([128, SB], f32, tag=f"acc{i}", name=f"acc{i}")
                       for i in range(6)]
                for dc in range(N_DC):
                    xf = xstage.tile([128, SB], f32, tag="xf")
                    nc.sync.dma_start(out=xf, in_=xT[dc * 128:(dc + 1) * 128, ss])
                    xr = xstage.tile([128, SB], f32r, tag="xr")
                    nc.vector.tensor_copy(out=xr, in_=xf)
                    st_flags = dict(start=(dc == 0), stop=(dc == N_DC - 1))
                    for h in range(G):
                        nc.tensor.matmul(
                            acc[h][:], wq_r[:, dc, h * 128:(h + 1) * 128], xr[:],
                            **st_flags)
                    nc.tensor.matmul(acc[4][:], wk_r[:, dc, :], xr[:], **st_flags)
                    nc.tensor.matmul(acc[5][:], wv_r[:, dc, :], xr[:], **st_flags)

                # RoPE for 4 q heads + k -> fp32r into persistent tiles
                for i in range(5):
                    dst = qr[i][:, ss] if i < G else kr[:, ss]
                    t_cos = rope.tile([128, SB], f32, tag="t_cos")
                    nc.vector.tensor_mul(t_cos, acc[i][:], cos_sb[:, ss])
                    t_plain = rope.tile([128, SB], f32, tag="t_plain")
                    nc.scalar.copy(out=t_plain, in_=acc[i][:])
                    t_swap = rope.tile([128, SB], f32, tag="t_swap")
                    nc.sync.dma_start(out=t_swap[0:64, :], in_=t_plain[64:128, :])
                    nc.sync.dma_start(out=t_swap[64:128, :], in_=t_plain[0:64, :])
                    nc.vector.tensor_mul(t_swap, t_swap, sinm_sb[:, ss])
                    nc.vector.tensor_add(dst, t_cos, t_swap)

                # V: copy PSUM->SBUF, PE-transpose 128x128 blocks to natural
                vt_sb = rope.tile([128, SB], f32, tag="vt_sb")
                nc.scalar.copy(out=vt_sb, in_=acc[5][:])
                for j in range(SB // 128):
                    tp = tpsum.tile([128, 128], f32, tag="tp")
                    nc.tensor.transpose(tp[:], vt_sb[:, j * 128:(j + 1) * 128],
                                        ident[:])
                    nc.scalar.copy(out=vnat[:, sb * 4 + j, :], in_=tp[:])

        # ================= Phase 2: attention ==============================
        inv_sqrt_dh = 1.0 / math.sqrt(Dh)
        with tc.tile_pool(name="wo_pool", bufs=1) as wo_pool, \
             tc.tile_pool(name="attn_pool", bufs=1) as attn_pool:

            # unnormalized attn^T per head [Dh, S], bf16
            attn = [attn_pool.tile([128, S], bf16, tag=f"attn{h}",
                                   name=f"attn{h}") for h in range(G)]
            # Wo arrives bf16 from the host; plain DMA, no rounding pass
            wo_r = wo_pool.tile([128, G, D], bf16, tag="wo_r")
            nc.sync.dma_start(out=wo_r,
                              in_=woT.rearrange("(h p) d -> p h d", p=128))

            with tc.tile_pool(name="expp", bufs=6) as expp, \
                 tc.tile_pool(name="zpool", bufs=2) as zpool, \
                 tc.tile_pool(name="scps", bufs=3, space="PSUM") as scps, \
                 tc.tile_pool(name="sumps", bufs=2, space="PSUM") as sumps, \
                 tc.tile_pool(name="pvps", bufs=2, space="PSUM") as pvps:
                for t in range(N_SB):
                    qs = slice(t * SB, (t + 1) * SB)
                    n_chunks = 4 * (t + 1)
                    for h in range(G):
                        pv_ps = pvps.tile([128, SB], f32, tag="pv")
                        sum_ps = sumps.tile([1, SB], f32, tag="sum")
                        sc_t = [None] * n_chunks
                        e_t = [None] * n_chunks

                        def emit_score(c):
                            sc = scps.tile([128, SB], f32, tag="sc", name="sc")
                            nc.tensor.matmul(
                                sc[:], kr[:, c * 128:(c + 1) * 128],
                                qr[h][:, qs], start=True, stop=True)
                            sc_t[c] = sc

                        def emit_exp(c):
                            e = expp.tile([128, SB], bf16, tag="e", name="e")
                            nc.scalar.activation(e[:], sc_t[c][:], AF.Exp,
                                                 scale=inv_sqrt_dh)
                            j = c - 4 * t
                            if j >= 0:  # chunk contains the causal diagonal
                                nc.vector.tensor_mul(
                                    e[:, j * 128:(j + 1) * 128],
                                    e[:, j * 128:(j + 1) * 128], trim_sb[:])
                                if j > 0:
                                    nc.vector.memset(e[:, 0:j * 128], 0.0)
                            e_t[c] = e

                        def emit_mm(c):
                            mmf = dict(start=(c == 0), stop=(c == n_chunks - 1))
                            e = e_t[c]
                            nc.tensor.matmul(sum_ps[:], ones_b[:], e[:], **mmf)
                            nc.tensor.matmul(pv_ps[:], vnat[:, c, :], e[:], **mmf)

                        # software pipeline: score(c+1) issues before mm(c)
                        emit_score(0)
                        emit_exp(0)
                        for c in range(1, n_chunks):
                            emit_score(c)
                            emit_exp(c)
                            emit_mm(c - 1)
                        emit_mm(n_chunks - 1)

                        z_sb = zpool.tile([1, SB], f32, tag="z")
                        nc.vector.tensor_copy(out=z_sb, in_=sum_ps[:])
                        rinv = zpool.tile([1, SB], f32, tag="rinv")
                        nc.vector.reciprocal(out=rinv, in_=z_sb)
                        rbc = zpool.tile([128, SB], f32, tag="rbc")
                        nc.gpsimd.partition_broadcast(rbc[:], rinv[:])
                        nc.vector.tensor_mul(attn[h][:, qs], pv_ps[:], rbc[:])

            # ============= Phase 3: output projection ======================
            with tc.tile_pool(name="obuf", bufs=3) as obuf, \
                 tc.tile_pool(name="ops", bufs=3, space="PSUM") as ops:
                for st in range(N_ST):
                    rs = slice(st * 128, (st + 1) * 128)
                    for nb in range(N_NB):
                        cs = slice(nb * SB, (nb + 1) * SB)
                        op = ops.tile([128, SB], f32, tag="op")
                        for h in range(G):
                            nc.tensor.matmul(op[:], attn[h][:, rs],
                                             wo_r[:, h, cs],
                                             start=(h == 0), stop=(h == G - 1))
                        ob = obuf.tile([128, SB], f32, tag="ob")
                        nc.scalar.copy(out=ob, in_=op[:])
                        nc.sync.dma_start(out=out[rs, cs], in_=ob)

    nc.compile()
    return nc


def _prep_inputs(hidden_states, Wq, Wk, Wv, Wo, cos, sin):
    x = np.asarray(hidden_states, dtype=np.float32).reshape(S, D)
    Wq = np.asarray(Wq, dtype=np.float32)
    Wk = np.asarray(Wk, dtype=np.float32)
    Wv = np.asarray(Wv, dtype=np.float32)
    Wo = np.asarray(Wo, dtype=np.float32)
    cos = np.asarray(cos, dtype=np.float32)
    sin = np.asarray(sin, dtype=np.float32)

    xT = np.ascontiguousarray(x.T)
    cosT = np.ascontiguousarray(cos.T)
    sinmT = np.ascontiguousarray(sin.T).copy()
    sinmT[: Dh // 2] *= -1.0
    trimask = np.triu(np.ones((128, 128), dtype=ml_dtypes.bfloat16))  # kpos<=q valid

    in_maps = []
    for c in range(N_CORES):
        in_maps.append({
            "xT": xT,
            "wqT": np.ascontiguousarray(Wq[c * EH:(c + 1) * EH, :].T),
            "wkT": np.ascontiguousarray(Wk[c * Dh:(c + 1) * Dh, :].T),
            "wvT": np.ascontiguousarray(Wv[c * Dh:(c + 1) * Dh, :].T),
            "woT": np.ascontiguousarray(
                Wo[:, c * EH:(c + 1) * EH].T).astype(ml_dtypes.bfloat16),
            "cosT": cosT,
            "sinmT": sinmT,
            "trim": trimask,
        })
    return in_maps


def run(trace=False, **inputs):
    """Run on hardware; returns (full_output, exec_time_ns or None)."""
    from concourse.bass_utils import run_bass_kernel_spmd

    if trace:
        _install_ntff_hook()
    if "nc" not in _cache:
        _cache["nc"] = _build()
    nc = _cache["nc"]
    in_maps = _prep_inputs(**inputs)
    res = run_bass_kernel_spmd(nc, in_maps, core_ids=list(range(N_CORES)),
                               trace=trace)
    acc = res.results[0]["out"].astype(np.float32)
    for c in range(1, N_CORES):
        acc += res.results[c]["out"]
    return acc.reshape(B, S, D), res.exec_time_ns


def _install_ntff_hook():
    """Register the axon NTFF profiling hook missing from this image."""
    import types
    try:
        import antenv
        from trn_agent_boot.trn_boot import _ntff_profile_via_ctypes
    except ImportError:
        return
    if "antenv.axon_hooks" in sys.modules:
        return
    mod = types.ModuleType("antenv.axon_hooks")
    mod._hook = _ntff_profile_via_ctypes("/opt/axon/libaxon_pjrt.so")
    mod.get_axon_ntff_profile_hook = lambda: mod._hook
    mod.set_axon_ntff_profile_hook = lambda h: setattr(mod, "_hook", h)
    sys.modules["antenv.axon_hooks"] = mod
    antenv.axon_hooks = mod


def kernel(**inputs):
    out, _ = run(trace=False, **inputs)
    return out


# revision 12
# speedup vs baseline: 1.2103x; 1.1778x over previous
"""GQA attention (BagleyAttention) on 8 Trainium2 NeuronCores.

Tensor-parallel over kv-head groups: core c owns kv head c and query heads
[4c, 4c+4). Each core computes its heads' attention and a partial output
projection [S, D]; the host sums the 8 partials.

Datapath is fp16 (10-bit mantissa, same as tf32) with fp32 PSUM
accumulation. Softmax exp uses a constant bias shift (softmax-invariant)
to keep exp weights inside fp16 range.
"""

import math
import sys

sys.path.insert(0, "/opt/trn_rl_repo")

import numpy as np

# Problem sizes (hardcoded per contract; kernel.py reads no sibling files).
B, S, D = 1, 2048, 4096
H, KV, Dh = 32, 8, 128
G = H // KV            # query heads per kv head (= per core)
EH = G * Dh            # per-core q projection width (512)
N_CORES = 8

SB = 512               # s-block width for projections / q-block width
N_SB = S // SB         # 4
N_DC = D // 128        # 32 d-chunks
N_ST = S // 128        # 16 s-tiles of 128
N_NB = D // SB         # 8 output d-blocks

EXP_BIAS = 9.5         # exp(s - EXP_BIAS); cancels in softmax normalization

_cache = {}


def _build():
    import concourse.bass as bass
    import concourse.mybir as mybir
    import concourse.tile as tile
    from concourse import bacc
    from concourse.masks import make_identity

    dt = mybir.dt
    f32, f16 = dt.float32, dt.float16
    AF = mybir.ActivationFunctionType

    nc = bacc.Bacc("TRN2", target_bir_lowering=False, debug=False)

    xT = nc.dram_tensor("xT", [D, S], f16, kind="ExternalInput").ap()
    wqT = nc.dram_tensor("wqT", [D, EH], f16, kind="ExternalInput").ap()
    wkT = nc.dram_tensor("wkT", [D, Dh], f16, kind="ExternalInput").ap()
    wvT = nc.dram_tensor("wvT", [D, Dh], f16, kind="ExternalInput").ap()
    woT = nc.dram_tensor("woT", [EH, D], f16, kind="ExternalInput").ap()
    cosT = nc.dram_tensor("cosT", [Dh, S], f32, kind="ExternalInput").ap()
    sinmT = nc.dram_tensor("sinmT", [Dh, S], f32, kind="ExternalInput").ap()
    maskadd = nc.dram_tensor("maskadd", [N_SB, 128, SB], f32,
                             kind="ExternalInput").ap()
    out = nc.dram_tensor("out", [S, D], f32, kind="ExternalOutput").ap()

    with tile.TileContext(nc) as tc, \
         tc.tile_pool(name="persist", bufs=1) as persist:
        # ---- long-lived tensors -------------------------------------------
        # RoPE'd projections, transposed layout [Dh, S], fp16
        qr = [persist.tile([128, S], f16, tag=f"qr{h}", name=f"qr{h}")
              for h in range(G)]
        kr = persist.tile([128, S], f16, tag="kr")
        # V in natural layout: [s-local 128, (s-tile, Dh)]
        vnat = persist.tile([128, N_ST, Dh], f16, tag="vnat")
        # small constants
        madd_sb = persist.tile([128, N_SB, SB], f32, tag="madd")
        ones_h = persist.tile([128, 1], f16, tag="ones")
        ident = persist.tile([128, 128], f32, tag="ident")
        ebias = persist.tile([128, 1], f32, tag="ebias")

        nc.sync.dma_start(out=madd_sb, in_=maskadd.rearrange("j p q -> p j q"))
        nc.vector.memset(ones_h, 1.0)
        nc.vector.memset(ebias, -EXP_BIAS)
        make_identity(nc, ident)

        # ================= Phase 1: projections + RoPE =====================
        with tc.tile_pool(name="wts", bufs=1) as wts, \
             tc.tile_pool(name="trig", bufs=1) as trig, \
             tc.tile_pool(name="xstage", bufs=6) as xstage, \
             tc.tile_pool(name="rope", bufs=2) as rope, \
             tc.tile_pool(name="p1psum", bufs=1, space="PSUM") as p1psum, \
             tc.tile_pool(name="tpsum", bufs=2, space="PSUM") as tpsum:

            # weights: [128(d-local), d-chunk, e] fp16, straight from DMA.
            wq_h = wts.tile([128, N_DC, EH], f16, tag="wq_h")
            wk_h = wts.tile([128, N_DC, Dh], f16, tag="wk_h")
            wv_h = wts.tile([128, N_DC, Dh], f16, tag="wv_h")
            wq_re = wqT.rearrange("(c p) e -> p c e", p=128)
            wk_re = wkT.rearrange("(c p) e -> p c e", p=128)
            wv_re = wvT.rearrange("(c p) e -> p c e", p=128)

            cos_sb = trig.tile([128, S], f32, tag="cos")
            sinm_sb = trig.tile([128, S], f32, tag="sinm")
            nc.sync.dma_start(out=cos_sb, in_=cosT)
            nc.sync.dma_start(out=sinm_sb, in_=sinmT)

            for sb in range(N_SB):
                ss = slice(sb * SB, (sb + 1) * SB)
                # 6 PSUM accumulators: 4 q heads + k + v, all [e=128, s=512]
                acc = [p1psum.tile([128, SB], f32, tag=f"acc{i}", name=f"acc{i}")
                       for i in range(6)]
                for dc in range(N_DC):
                    if sb == 0:
                        # weight chunk loads interleaved with first s-block
                        nc.sync.dma_start(out=wq_h[:, dc, :], in_=wq_re[:, dc, :])
                        nc.sync.dma_start(out=wk_h[:, dc, :], in_=wk_re[:, dc, :])
                        nc.sync.dma_start(out=wv_h[:, dc, :], in_=wv_re[:, dc, :])
                    xf = xstage.tile([128, SB], f16, tag="xf")
                    nc.sync.dma_start(out=xf, in_=xT[dc * 128:(dc + 1) * 128, ss])
                    st_flags = dict(start=(dc == 0), stop=(dc == N_DC - 1))
                    for h in range(G):
                        nc.tensor.matmul(
                            acc[h][:], wq_h[:, dc, h * 128:(h + 1) * 128], xf[:],
                            **st_flags)
                    nc.tensor.matmul(acc[4][:], wk_h[:, dc, :], xf[:], **st_flags)
                    nc.tensor.matmul(acc[5][:], wv_h[:, dc, :], xf[:], **st_flags)

                # RoPE for 4 q heads + k -> fp16 into persistent tiles
                for i in range(5):
                    dst = qr[i][:, ss] if i < G else kr[:, ss]
                    t_cos = rope.tile([128, SB], f32, tag="t_cos")
                    nc.vector.tensor_mul(t_cos, acc[i][:], cos_sb[:, ss])
                    t_plain = rope.tile([128, SB], f32, tag="t_plain")
                    nc.scalar.copy(out=t_plain, in_=acc[i][:])
                    t_swap = rope.tile([128, SB], f32, tag="t_swap")
                    nc.sync.dma_start(out=t_swap[0:64, :], in_=t_plain[64:128, :])
                    nc.sync.dma_start(out=t_swap[64:128, :], in_=t_plain[0:64, :])
                    nc.vector.tensor_mul(t_swap, t_swap, sinm_sb[:, ss])
                    nc.vector.tensor_add(dst, t_cos, t_swap)

                # V: copy PSUM->SBUF, PE-transpose 128x128 blocks to natural
                vt_sb = rope.tile([128, SB], f32, tag="vt_sb")
                nc.scalar.copy(out=vt_sb, in_=acc[5][:])
                for j in range(SB // 128):
                    tp = tpsum.tile([128, 128], f32, tag="tp")
                    nc.tensor.transpose(tp[:], vt_sb[:, j * 128:(j + 1) * 128],
                                        ident[:])
                    nc.scalar.copy(out=vnat[:, sb * 4 + j, :], in_=tp[:])

        # ================= Phase 2: attention ==============================
        inv_sqrt_dh = 1.0 / math.sqrt(Dh)
        with tc.tile_pool(name="wo_pool", bufs=1) as wo_pool, \
             tc.tile_pool(name="attn_pool", bufs=1) as attn_pool:

            # unnormalized attn^T per head [Dh, S], fp16
            attn = [attn_pool.tile([128, S], f16, tag=f"attn{h}",
                                   name=f"attn{h}") for h in range(G)]
            # Wo arrives fp16 from the host; plain DMA
            wo_r = wo_pool.tile([128, G, D], f16, tag="wo_r")
            nc.sync.dma_start(out=wo_r,
                              in_=woT.rearrange("(h p) d -> p h d", p=128))

            with tc.tile_pool(name="expp", bufs=6) as expp, \
                 tc.tile_pool(name="zpool", bufs=2) as zpool, \
                 tc.tile_pool(name="scps", bufs=3, space="PSUM") as scps, \
                 tc.tile_pool(name="sumps", bufs=2, space="PSUM") as sumps, \
                 tc.tile_pool(name="pvps", bufs=2, space="PSUM") as pvps:
                for t in range(N_SB):
                    qs = slice(t * SB, (t + 1) * SB)
                    n_chunks = 4 * (t + 1)
                    for h in range(G):
                        pv_ps = pvps.tile([128, SB], f32, tag="pv")
                        sum_ps = sumps.tile([1, SB], f32, tag="sum")
                        sc_t = [None] * n_chunks
                        e_t = [None] * n_chunks

                        def emit_score(c):
                            sc = scps.tile([128, SB], f32, tag="sc", name="sc")
                            nc.tensor.matmul(
                                sc[:], kr[:, c * 128:(c + 1) * 128],
                                qr[h][:, qs], start=True, stop=True)
                            sc_t[c] = sc

                        def emit_exp(c):
                            j = c - 4 * t
                            if j >= 0:  # chunk contains the causal diagonal
                                nc.vector.tensor_add(sc_t[c][:], sc_t[c][:],
                                                     madd_sb[:, j, :])
                            e = expp.tile([128, SB], f16, tag="e", name="e")
                            nc.scalar.activation(e[:], sc_t[c][:], AF.Exp,
                                                 scale=inv_sqrt_dh,
                                                 bias=ebias[:])
                            e_t[c] = e

                        def emit_mm(c):
                            mmf = dict(start=(c == 0), stop=(c == n_chunks - 1))
                            e = e_t[c]
                            nc.tensor.matmul(sum_ps[:], ones_h[:], e[:], **mmf)
                            nc.tensor.matmul(pv_ps[:], vnat[:, c, :], e[:], **mmf)

                        # software pipeline: score(c+1) issues before mm(c)
                        emit_score(0)
                        emit_exp(0)
                        for c in range(1, n_chunks):
                            emit_score(c)
                            emit_exp(c)
                            emit_mm(c - 1)
                        emit_mm(n_chunks - 1)

                        z_sb = zpool.tile([1, SB], f32, tag="z")
                        nc.vector.tensor_copy(out=z_sb, in_=sum_ps[:])
                        rinv = zpool.tile([1, SB], f32, tag="rinv")
                        nc.vector.reciprocal(out=rinv, in_=z_sb)
                        rbc = zpool.tile([128, SB], f32, tag="rbc")
                        nc.gpsimd.partition_broadcast(rbc[:], rinv[:])
                        nc.vector.tensor_mul(attn[h][:, qs], pv_ps[:], rbc[:])

            # ============= Phase 3: output projection ======================
            with tc.tile_pool(name="obuf", bufs=3) as obuf, \
                 tc.tile_pool(name="ops", bufs=3, space="PSUM") as ops:
                for st in range(N_ST):
                    rs = slice(st * 128, (st + 1) * 128)
                    for nb in range(N_NB):
                        cs = slice(nb * SB, (nb + 1) * SB)
                        op = ops.tile([128, SB], f32, tag="op")
                        for h in range(G):
                            nc.tensor.matmul(op[:], attn[h][:, rs],
                                             wo_r[:, h, cs],
                                             start=(h == 0), stop=(h == G - 1))
                        ob = obuf.tile([128, SB], f32, tag="ob")
                        nc.scalar.copy(out=ob, in_=op[:])
                        nc.sync.dma_start(out=out[rs, cs], in_=ob)

    nc.compile()
    return nc


def _prep_inputs(hidden_states, Wq, Wk, Wv, Wo, cos, sin):
    x = np.asarray(hidden_states, dtype=np.float32).reshape(S, D)
    Wq = np.asarray(Wq, dtype=np.float32)
    Wk = np.asarray(Wk, dtype=np.float32)
    Wv = np.asarray(Wv, dtype=np.float32)
    Wo = np.asarray(Wo, dtype=np.float32)
    cos = np.asarray(cos, dtype=np.float32)
    sin = np.asarray(sin, dtype=np.float32)

    xT = np.ascontiguousarray(x.T).astype(np.float16)
    cosT = np.ascontiguousarray(cos.T)
    sinmT = np.ascontiguousarray(sin.T).copy()
    sinmT[: Dh // 2] *= -1.0
    # additive causal masks for the 4 diagonal offsets, applied to raw
    # (pre-scale) scores in PSUM: -1e4 where kpos > q.
    maskadd = np.zeros((N_SB, 128, SB), dtype=np.float32)
    kp = np.arange(128)[:, None]
    qc = np.arange(SB)[None, :]
    for j in range(N_SB):
        maskadd[j] = np.where(kp + 128 * j > qc, -1e4, 0.0).astype(np.float32)

    in_maps = []
    for c in range(N_CORES):
        in_maps.append({
            "xT": xT,
            "wqT": np.ascontiguousarray(
                Wq[c * EH:(c + 1) * EH, :].T).astype(np.float16),
            "wkT": np.ascontiguousarray(
                Wk[c * Dh:(c + 1) * Dh, :].T).astype(np.float16),
            "wvT": np.ascontiguousarray(
                Wv[c * Dh:(c + 1) * Dh, :].T).astype(np.float16),
            "woT": np.ascontiguousarray(
                Wo[:, c * EH:(c + 1) * EH].T).astype(np.float16),
            "cosT": cosT,
            "sinmT": sinmT,
            "maskadd": maskadd,
        })
    return in_maps


def run(trace=False, **inputs):
    """Run on hardware; returns (full_output, exec_time_ns or None)."""
    from concourse.bass_utils import run_bass_kernel_spmd

    if trace:
        _install_ntff_hook()
    if "nc" not in _cache:
        _cache["nc"] = _build()
    nc = _cache["nc"]
    in_maps = _prep_inputs(**inputs)
    res = run_bass_kernel_spmd(nc, in_maps, core_ids=list(range(N_CORES)),
                               trace=trace)
    acc = res.results[0]["out"].astype(np.float32)
    for c in range(1, N_CORES):
        acc += res.results[c]["out"]
    return acc.reshape(B, S, D), res.exec_time_ns


def _install_ntff_hook():
    """Register the axon NTFF profiling hook missing from this image."""
    import types
    try:
        import antenv
        from trn_agent_boot.trn_boot import _ntff_profile_via_ctypes
    except ImportError:
        return
    if "antenv.axon_hooks" in sys.modules:
        return
    mod = types.ModuleType("antenv.axon_hooks")
    mod._hook = _ntff_profile_via_ctypes("/opt/axon/libaxon_pjrt.so")
    mod.get_axon_ntff_profile_hook = lambda: mod._hook
    mod.set_axon_ntff_profile_hook = lambda h: setattr(mod, "_hook", h)
    sys.modules["antenv.axon_hooks"] = mod
    antenv.axon_hooks = mod


def kernel(**inputs):
    out, _ = run(trace=False, **inputs)
    return out


# revision 13
# speedup vs baseline: 1.2790x; 1.0568x over previous
"""GQA attention (BagleyAttention) on 8 Trainium2 NeuronCores.

Tensor-parallel over kv-head groups: core c owns kv head c and query heads
[4c, 4c+4). Each core computes its heads' attention and a partial output
projection [S, D]; the host sums the 8 partials.

Datapath is fp16 (10-bit mantissa, same as tf32) with fp32 PSUM
accumulation. Softmax exp uses a constant bias shift (softmax-invariant)
to keep exp weights inside fp16 range.
"""

import math
import sys

sys.path.insert(0, "/opt/trn_rl_repo")

import numpy as np

# Problem sizes (hardcoded per contract; kernel.py reads no sibling files).
B, S, D = 1, 2048, 4096
H, KV, Dh = 32, 8, 128
G = H // KV            # query heads per kv head (= per core)
EH = G * Dh            # per-core q projection width (512)
N_CORES = 8

SB = 512               # s-block width for projections / q-block width
N_SB = S // SB         # 4
N_DC = D // 128        # 32 d-chunks
N_ST = S // 128        # 16 s-tiles of 128
N_NB = D // SB         # 8 output d-blocks

EXP_BIAS = 9.5         # exp(s - EXP_BIAS); cancels in softmax normalization

_cache = {}


def _build():
    import concourse.bass as bass
    import concourse.mybir as mybir
    import concourse.tile as tile
    from concourse import bacc
    from concourse.masks import make_identity

    dt = mybir.dt
    f32, f16 = dt.float32, dt.float16
    AF = mybir.ActivationFunctionType

    nc = bacc.Bacc("TRN2", target_bir_lowering=False, debug=False)

    xT = nc.dram_tensor("xT", [D, S], f16, kind="ExternalInput").ap()
    wqT = nc.dram_tensor("wqT", [D, EH], f16, kind="ExternalInput").ap()
    wkT = nc.dram_tensor("wkT", [D, Dh], f16, kind="ExternalInput").ap()
    wvT = nc.dram_tensor("wvT", [D, Dh], f16, kind="ExternalInput").ap()
    woT = nc.dram_tensor("woT", [EH, D], f16, kind="ExternalInput").ap()
    cosT = nc.dram_tensor("cosT", [Dh, S], f32, kind="ExternalInput").ap()
    sinmT = nc.dram_tensor("sinmT", [Dh, S], f32, kind="ExternalInput").ap()
    maskadd = nc.dram_tensor("maskadd", [N_SB, 128, SB], f32,
                             kind="ExternalInput").ap()
    out = nc.dram_tensor("out", [S, D], f32, kind="ExternalOutput").ap()

    with tile.TileContext(nc) as tc, \
         tc.tile_pool(name="persist", bufs=1) as persist:
        # ---- long-lived tensors -------------------------------------------
        # RoPE'd projections, transposed layout [Dh, S], fp16
        qr = [persist.tile([128, S], f16, tag=f"qr{h}", name=f"qr{h}")
              for h in range(G)]
        kr = persist.tile([128, S], f16, tag="kr")
        # V in natural layout: [s-local 128, (s-tile, Dh)]
        vnat = persist.tile([128, N_ST, Dh], f16, tag="vnat")
        # small constants
        madd_sb = persist.tile([128, N_SB, SB], f32, tag="madd")
        ones_h = persist.tile([128, 1], f16, tag="ones")
        ident = persist.tile([128, 128], f32, tag="ident")
        ebias = persist.tile([128, 1], f32, tag="ebias")

        nc.sync.dma_start(out=madd_sb, in_=maskadd.rearrange("j p q -> p j q"))
        nc.vector.memset(ones_h, 1.0)
        nc.vector.memset(ebias, -EXP_BIAS)
        make_identity(nc, ident)

        # ================= Phase 1: projections + RoPE =====================
        with tc.tile_pool(name="wts", bufs=1) as wts, \
             tc.tile_pool(name="trig", bufs=1) as trig, \
             tc.tile_pool(name="xstage", bufs=6) as xstage, \
             tc.tile_pool(name="rope", bufs=2) as rope, \
             tc.tile_pool(name="p1psum", bufs=1, space="PSUM") as p1psum, \
             tc.tile_pool(name="tpsum", bufs=2, space="PSUM") as tpsum:

            # weights: [128(d-local), d-chunk, e] fp16, straight from DMA.
            wq_h = wts.tile([128, N_DC, EH], f16, tag="wq_h")
            wk_h = wts.tile([128, N_DC, Dh], f16, tag="wk_h")
            wv_h = wts.tile([128, N_DC, Dh], f16, tag="wv_h")
            wq_re = wqT.rearrange("(c p) e -> p c e", p=128)
            wk_re = wkT.rearrange("(c p) e -> p c e", p=128)
            wv_re = wvT.rearrange("(c p) e -> p c e", p=128)

            cos_sb = trig.tile([128, S], f32, tag="cos")
            sinm_sb = trig.tile([128, S], f32, tag="sinm")
            nc.sync.dma_start(out=cos_sb, in_=cosT)
            nc.sync.dma_start(out=sinm_sb, in_=sinmT)

            for sb in range(N_SB):
                ss = slice(sb * SB, (sb + 1) * SB)
                # 6 PSUM accumulators: 4 q heads + k + v, all [e=128, s=512]
                acc = [p1psum.tile([128, SB], f32, tag=f"acc{i}", name=f"acc{i}")
                       for i in range(6)]
                for dc in range(N_DC):
                    if sb == 0:
                        # weight chunk loads interleaved with first s-block
                        nc.sync.dma_start(out=wq_h[:, dc, :], in_=wq_re[:, dc, :])
                        nc.sync.dma_start(out=wk_h[:, dc, :], in_=wk_re[:, dc, :])
                        nc.sync.dma_start(out=wv_h[:, dc, :], in_=wv_re[:, dc, :])
                    xf = xstage.tile([128, SB], f16, tag="xf")
                    nc.sync.dma_start(out=xf, in_=xT[dc * 128:(dc + 1) * 128, ss])
                    st_flags = dict(start=(dc == 0), stop=(dc == N_DC - 1))
                    for h in range(G):
                        nc.tensor.matmul(
                            acc[h][:], wq_h[:, dc, h * 128:(h + 1) * 128], xf[:],
                            **st_flags)
                    nc.tensor.matmul(acc[4][:], wk_h[:, dc, :], xf[:], **st_flags)
                    nc.tensor.matmul(acc[5][:], wv_h[:, dc, :], xf[:], **st_flags)

                # Drain all 6 PSUM accumulators first (ACT copies + DVE
                # cos-muls read them in parallel) so the next s-block's
                # matmuls can start; then finish RoPE from the SBUF copies.
                t_plain = [rope.tile([128, SB], f32, tag=f"t_plain{i}",
                                     name=f"t_plain{i}") for i in range(5)]
                t_cos = [rope.tile([128, SB], f32, tag=f"t_cos{i}",
                                   name=f"t_cos{i}") for i in range(5)]
                vt_sb = rope.tile([128, SB], f32, tag="vt_sb")
                for i in range(5):
                    nc.scalar.copy(out=t_plain[i], in_=acc[i][:])
                    nc.vector.tensor_mul(t_cos[i], acc[i][:], cos_sb[:, ss])
                nc.scalar.copy(out=vt_sb, in_=acc[5][:])

                for i in range(5):
                    dst = qr[i][:, ss] if i < G else kr[:, ss]
                    t_swap = rope.tile([128, SB], f32, tag="t_swap")
                    nc.sync.dma_start(out=t_swap[0:64, :],
                                      in_=t_plain[i][64:128, :])
                    nc.sync.dma_start(out=t_swap[64:128, :],
                                      in_=t_plain[i][0:64, :])
                    nc.vector.tensor_mul(t_swap, t_swap, sinm_sb[:, ss])
                    nc.vector.tensor_add(dst, t_cos[i], t_swap)

                # V: PE-transpose 128x128 blocks to natural layout
                for j in range(SB // 128):
                    tp = tpsum.tile([128, 128], f32, tag="tp")
                    nc.tensor.transpose(tp[:], vt_sb[:, j * 128:(j + 1) * 128],
                                        ident[:])
                    nc.scalar.copy(out=vnat[:, sb * 4 + j, :], in_=tp[:])

        # ================= Phase 2: attention ==============================
        inv_sqrt_dh = 1.0 / math.sqrt(Dh)
        with tc.tile_pool(name="wo_pool", bufs=1) as wo_pool, \
             tc.tile_pool(name="attn_pool", bufs=1) as attn_pool:

            # unnormalized attn^T per head [Dh, S], fp16
            attn = [attn_pool.tile([128, S], f16, tag=f"attn{h}",
                                   name=f"attn{h}") for h in range(G)]
            # Wo tile (loaded after the first q-block to keep the
            # phase-1 ramp's DMA bandwidth free; first use is phase 3)
            wo_r = wo_pool.tile([128, G, D], f16, tag="wo_r")

            with tc.tile_pool(name="expp", bufs=8) as expp, \
                 tc.tile_pool(name="zpool", bufs=2) as zpool, \
                 tc.tile_pool(name="scps", bufs=3, space="PSUM") as scps, \
                 tc.tile_pool(name="sumps", bufs=2, space="PSUM") as sumps, \
                 tc.tile_pool(name="pvps", bufs=2, space="PSUM") as pvps:

                tasks = [(t, h) for t in range(N_SB) for h in range(G)]
                state = {}   # (t,h) -> dict(pv, sum, e, n)

                def ensure_state(t, h):
                    if (t, h) not in state:
                        n = 4 * (t + 1)
                        state[(t, h)] = dict(
                            pv=pvps.tile([128, SB], f32, tag="pv", name="pv"),
                            sum=sumps.tile([1, SB], f32, tag="sum", name="sum"),
                            e=[None] * n, n=n)
                    return state[(t, h)]

                def emit_score_exp(t, h, c):
                    st = ensure_state(t, h)
                    qs = slice(t * SB, (t + 1) * SB)
                    sc = scps.tile([128, SB], f32, tag="sc", name="sc")
                    nc.tensor.matmul(sc[:], kr[:, c * 128:(c + 1) * 128],
                                     qr[h][:, qs], start=True, stop=True)
                    j = c - 4 * t
                    if j >= 0:  # chunk contains the causal diagonal
                        nc.vector.tensor_add(sc[:], sc[:], madd_sb[:, j, :])
                    e = expp.tile([128, SB], f16, tag="e", name="e")
                    nc.scalar.activation(e[:], sc[:], AF.Exp,
                                         scale=inv_sqrt_dh, bias=ebias[:])
                    st["e"][c] = e

                def emit_mm(t, h, c):
                    st = state[(t, h)]
                    mmf = dict(start=(c == 0), stop=(c == st["n"] - 1))
                    e = st["e"][c]
                    nc.tensor.matmul(st["sum"][:], ones_h[:], e[:], **mmf)
                    nc.tensor.matmul(st["pv"][:], vnat[:, c, :], e[:], **mmf)
                    st["e"][c] = None
                    if c == st["n"] - 1:
                        emit_epilogue(t, h)

                def emit_epilogue(t, h):
                    st = state.pop((t, h))
                    qs = slice(t * SB, (t + 1) * SB)
                    z_sb = zpool.tile([1, SB], f32, tag="z", name="z")
                    nc.vector.tensor_copy(out=z_sb, in_=st["sum"][:])
                    rinv = zpool.tile([1, SB], f32, tag="rinv", name="rinv")
                    nc.vector.reciprocal(out=rinv, in_=z_sb)
                    rbc = zpool.tile([128, SB], f32, tag="rbc", name="rbc")
                    nc.gpsimd.partition_broadcast(rbc[:], rinv[:])
                    nc.vector.tensor_mul(attn[h][:, qs], st["pv"][:], rbc[:])

                work = [(t, h, c) for (t, h) in tasks
                        for c in range(4 * (t + 1))]
                prev = None
                for k, (t, h, c) in enumerate(work):
                    emit_score_exp(t, h, c)
                    if prev is not None:
                        emit_mm(*prev)
                    prev = (t, h, c)
                    if k == 3:  # after the first q-block is in flight
                        nc.sync.dma_start(
                            out=wo_r,
                            in_=woT.rearrange("(h p) d -> p h d", p=128))
                emit_mm(*prev)

            # ============= Phase 3: output projection ======================
            with tc.tile_pool(name="obuf", bufs=3) as obuf, \
                 tc.tile_pool(name="ops", bufs=3, space="PSUM") as ops:
                for st in range(N_ST):
                    rs = slice(st * 128, (st + 1) * 128)
                    for nb in range(N_NB):
                        cs = slice(nb * SB, (nb + 1) * SB)
                        op = ops.tile([128, SB], f32, tag="op")
                        for h in range(G):
                            nc.tensor.matmul(op[:], attn[h][:, rs],
                                             wo_r[:, h, cs],
                                             start=(h == 0), stop=(h == G - 1))
                        ob = obuf.tile([128, SB], f32, tag="ob")
                        nc.scalar.copy(out=ob, in_=op[:])
                        nc.sync.dma_start(out=out[rs, cs], in_=ob)

    nc.compile()
    return nc


def _prep_inputs(hidden_states, Wq, Wk, Wv, Wo, cos, sin):
    x = np.asarray(hidden_states, dtype=np.float32).reshape(S, D)
    Wq = np.asarray(Wq, dtype=np.float32)
    Wk = np.asarray(Wk, dtype=np.float32)
    Wv = np.asarray(Wv, dtype=np.float32)
    Wo = np.asarray(Wo, dtype=np.float32)
    cos = np.asarray(cos, dtype=np.float32)
    sin = np.asarray(sin, dtype=np.float32)

    xT = np.ascontiguousarray(x.T).astype(np.float16)
    cosT = np.ascontiguousarray(cos.T)
    sinmT = np.ascontiguousarray(sin.T).copy()
    sinmT[: Dh // 2] *= -1.0
    # additive causal masks for the 4 diagonal offsets, applied to raw
    # (pre-scale) scores in PSUM: -1e4 where kpos > q.
    maskadd = np.zeros((N_SB, 128, SB), dtype=np.float32)
    kp = np.arange(128)[:, None]
    qc = np.arange(SB)[None, :]
    for j in range(N_SB):
        maskadd[j] = np.where(kp + 128 * j > qc, -1e4, 0.0).astype(np.float32)

    in_maps = []
    for c in range(N_CORES):
        in_maps.append({
            "xT": xT,
            "wqT": np.ascontiguousarray(
                Wq[c * EH:(c + 1) * EH, :].T).astype(np.float16),
            "wkT": np.ascontiguousarray(
                Wk[c * Dh:(c + 1) * Dh, :].T).astype(np.float16),
            "wvT": np.ascontiguousarray(
                Wv[c * Dh:(c + 1) * Dh, :].T).astype(np.float16),
            "woT": np.ascontiguousarray(
                Wo[:, c * EH:(c + 1) * EH].T).astype(np.float16),
            "cosT": cosT,
            "sinmT": sinmT,
            "maskadd": maskadd,
        })
    return in_maps


def run(trace=False, **inputs):
    """Run on hardware; returns (full_output, exec_time_ns or None)."""
    from concourse.bass_utils import run_bass_kernel_spmd

    if trace:
        _install_ntff_hook()
    if "nc" not in _cache:
        _cache["nc"] = _build()
    nc = _cache["nc"]
    in_maps = _prep_inputs(**inputs)
    res = run_bass_kernel_spmd(nc, in_maps, core_ids=list(range(N_CORES)),
                               trace=trace)
    acc = res.results[0]["out"].astype(np.float32)
    for c in range(1, N_CORES):
        acc += res.results[c]["out"]
    return acc.reshape(B, S, D), res.exec_time_ns


def _install_ntff_hook():
    """Register the axon NTFF profiling hook missing from this image."""
    import types
    try:
        import antenv
        from trn_agent_boot.trn_boot import _ntff_profile_via_ctypes
    except ImportError:
        return
    if "antenv.axon_hooks" in sys.modules:
        return
    mod = types.ModuleType("antenv.axon_hooks")
    mod._hook = _ntff_profile_via_ctypes("/opt/axon/libaxon_pjrt.so")
    mod.get_axon_ntff_profile_hook = lambda: mod._hook
    mod.set_axon_ntff_profile_hook = lambda h: setattr(mod, "_hook", h)
    sys.modules["antenv.axon_hooks"] = mod
    antenv.axon_hooks = mod


def kernel(**inputs):
    out, _ = run(trace=False, **inputs)
    return out


# revision 14
# speedup vs baseline: 1.3098x; 1.0241x over previous
"""GQA attention (BagleyAttention) on 8 Trainium2 NeuronCores.

Tensor-parallel over kv-head groups: core c owns kv head c and query heads
[4c, 4c+4). Each core computes its heads' attention and a partial output
projection [S, D]; the host sums the 8 partials.

Datapath is fp16 (10-bit mantissa, same as tf32) with fp32 PSUM
accumulation. Softmax exp uses a constant bias shift (softmax-invariant)
to keep exp weights inside fp16 range.
"""

import math
import sys

sys.path.insert(0, "/opt/trn_rl_repo")

import numpy as np

# Problem sizes (hardcoded per contract; kernel.py reads no sibling files).
B, S, D = 1, 2048, 4096
H, KV, Dh = 32, 8, 128
G = H // KV            # query heads per kv head (= per core)
EH = G * Dh            # per-core q projection width (512)
N_CORES = 8

SB = 512               # s-block width for projections / q-block width
N_SB = S // SB         # 4
N_DC = D // 128        # 32 d-chunks
N_ST = S // 128        # 16 s-tiles of 128
N_NB = D // SB         # 8 output d-blocks

EXP_BIAS = 9.5         # exp(s - EXP_BIAS); cancels in softmax normalization

_cache = {}


def _build():
    import concourse.bass as bass
    import concourse.mybir as mybir
    import concourse.tile as tile
    from concourse import bacc
    from concourse.masks import make_identity

    dt = mybir.dt
    f32, f16 = dt.float32, dt.float16
    AF = mybir.ActivationFunctionType

    nc = bacc.Bacc("TRN2", target_bir_lowering=False, debug=False)

    xT = nc.dram_tensor("xT", [D, S], f16, kind="ExternalInput").ap()
    wqT = nc.dram_tensor("wqT", [D, EH], f16, kind="ExternalInput").ap()
    wkT = nc.dram_tensor("wkT", [D, Dh], f16, kind="ExternalInput").ap()
    wvT = nc.dram_tensor("wvT", [D, Dh], f16, kind="ExternalInput").ap()
    woT = nc.dram_tensor("woT", [EH, D], f16, kind="ExternalInput").ap()
    cosT = nc.dram_tensor("cosT", [Dh, S], f32, kind="ExternalInput").ap()
    sinmT = nc.dram_tensor("sinmT", [Dh, S], f32, kind="ExternalInput").ap()
    maskadd = nc.dram_tensor("maskadd", [N_SB, 128, SB], f32,
                             kind="ExternalInput").ap()
    out = nc.dram_tensor("out", [S, D], f32, kind="ExternalOutput").ap()

    with tile.TileContext(nc) as tc, \
         tc.tile_pool(name="persist", bufs=1) as persist:
        # ---- long-lived tensors -------------------------------------------
        # RoPE'd projections, transposed layout [Dh, S], fp16
        qr = [persist.tile([128, S], f16, tag=f"qr{h}", name=f"qr{h}")
              for h in range(G)]
        kr = persist.tile([128, S], f16, tag="kr")
        # V in natural layout: [s-local 128, (s-tile, Dh)]
        vnat = persist.tile([128, N_ST, Dh], f16, tag="vnat")
        # small constants
        madd_sb = persist.tile([128, N_SB, SB], f32, tag="madd")
        ones_h = persist.tile([128, 128], f16, tag="ones")
        ident = persist.tile([128, 128], f32, tag="ident")
        ebias = persist.tile([128, 1], f32, tag="ebias")

        nc.sync.dma_start(out=madd_sb, in_=maskadd.rearrange("j p q -> p j q"))
        nc.vector.memset(ones_h, 1.0)
        nc.vector.memset(ebias, -EXP_BIAS)
        make_identity(nc, ident)

        # ================= Phase 1: projections + RoPE =====================
        with tc.tile_pool(name="wts", bufs=1) as wts, \
             tc.tile_pool(name="trig", bufs=1) as trig, \
             tc.tile_pool(name="xstage", bufs=6) as xstage, \
             tc.tile_pool(name="rope", bufs=2) as rope, \
             tc.tile_pool(name="p1psum", bufs=1, space="PSUM") as p1psum, \
             tc.tile_pool(name="tpsum", bufs=2, space="PSUM") as tpsum:

            # weights: [128(d-local), d-chunk, e] fp16, straight from DMA.
            wq_h = wts.tile([128, N_DC, EH], f16, tag="wq_h")
            wk_h = wts.tile([128, N_DC, Dh], f16, tag="wk_h")
            wv_h = wts.tile([128, N_DC, Dh], f16, tag="wv_h")
            wq_re = wqT.rearrange("(c p) e -> p c e", p=128)
            wk_re = wkT.rearrange("(c p) e -> p c e", p=128)
            wv_re = wvT.rearrange("(c p) e -> p c e", p=128)

            cos_sb = trig.tile([128, S], f32, tag="cos")
            sinm_sb = trig.tile([128, S], f32, tag="sinm")
            nc.sync.dma_start(out=cos_sb, in_=cosT)
            nc.sync.dma_start(out=sinm_sb, in_=sinmT)

            for sb in range(N_SB):
                ss = slice(sb * SB, (sb + 1) * SB)
                # 6 PSUM accumulators: 4 q heads + k + v, all [e=128, s=512]
                acc = [p1psum.tile([128, SB], f32, tag=f"acc{i}", name=f"acc{i}")
                       for i in range(6)]
                for dc in range(N_DC):
                    if sb == 0:
                        # weight chunk loads interleaved with first s-block
                        nc.sync.dma_start(out=wq_h[:, dc, :], in_=wq_re[:, dc, :])
                        nc.sync.dma_start(out=wk_h[:, dc, :], in_=wk_re[:, dc, :])
                        nc.sync.dma_start(out=wv_h[:, dc, :], in_=wv_re[:, dc, :])
                    xf = xstage.tile([128, SB], f16, tag="xf")
                    nc.sync.dma_start(out=xf, in_=xT[dc * 128:(dc + 1) * 128, ss])
                    st_flags = dict(start=(dc == 0), stop=(dc == N_DC - 1))
                    for h in range(G):
                        nc.tensor.matmul(
                            acc[h][:], wq_h[:, dc, h * 128:(h + 1) * 128], xf[:],
                            **st_flags)
                    nc.tensor.matmul(acc[4][:], wk_h[:, dc, :], xf[:], **st_flags)
                    nc.tensor.matmul(acc[5][:], wv_h[:, dc, :], xf[:], **st_flags)

                # Drain all 6 PSUM accumulators first (ACT copies + DVE
                # cos-muls read them in parallel) so the next s-block's
                # matmuls can start; then finish RoPE from the SBUF copies.
                t_plain = [rope.tile([128, SB], f32, tag=f"t_plain{i}",
                                     name=f"t_plain{i}") for i in range(5)]
                t_cos = [rope.tile([128, SB], f32, tag=f"t_cos{i}",
                                   name=f"t_cos{i}") for i in range(5)]
                vt_sb = rope.tile([128, SB], f32, tag="vt_sb")
                for i in range(5):
                    nc.scalar.copy(out=t_plain[i], in_=acc[i][:])
                    nc.vector.tensor_mul(t_cos[i], acc[i][:], cos_sb[:, ss])
                nc.scalar.copy(out=vt_sb, in_=acc[5][:])

                for i in range(5):
                    dst = qr[i][:, ss] if i < G else kr[:, ss]
                    t_swap = rope.tile([128, SB], f32, tag="t_swap")
                    nc.sync.dma_start(out=t_swap[0:64, :],
                                      in_=t_plain[i][64:128, :])
                    nc.sync.dma_start(out=t_swap[64:128, :],
                                      in_=t_plain[i][0:64, :])
                    nc.vector.tensor_mul(t_swap, t_swap, sinm_sb[:, ss])
                    nc.vector.tensor_add(dst, t_cos[i], t_swap)

                # V: PE-transpose 128x128 blocks to natural layout
                for j in range(SB // 128):
                    tp = tpsum.tile([128, 128], f32, tag="tp")
                    nc.tensor.transpose(tp[:], vt_sb[:, j * 128:(j + 1) * 128],
                                        ident[:])
                    nc.scalar.copy(out=vnat[:, sb * 4 + j, :], in_=tp[:])

        # ================= Phase 2: attention ==============================
        inv_sqrt_dh = 1.0 / math.sqrt(Dh)
        with tc.tile_pool(name="wo_pool", bufs=1) as wo_pool, \
             tc.tile_pool(name="attn_pool", bufs=1) as attn_pool:

            # unnormalized attn^T per head [Dh, S], fp16
            attn = [attn_pool.tile([128, S], f16, tag=f"attn{h}",
                                   name=f"attn{h}") for h in range(G)]
            # Wo tile (loaded after the first q-block to keep the
            # phase-1 ramp's DMA bandwidth free; first use is phase 3)
            wo_r = wo_pool.tile([128, G, D], f16, tag="wo_r")

            with tc.tile_pool(name="expp", bufs=8) as expp, \
                 tc.tile_pool(name="zpool", bufs=2) as zpool, \
                 tc.tile_pool(name="scps", bufs=3, space="PSUM") as scps, \
                 tc.tile_pool(name="sumps", bufs=2, space="PSUM") as sumps, \
                 tc.tile_pool(name="pvps", bufs=2, space="PSUM") as pvps:

                tasks = [(t, h) for t in range(N_SB) for h in range(G)]
                state = {}   # (t,h) -> dict(pv, sum, e, n)

                def ensure_state(t, h):
                    if (t, h) not in state:
                        n = 4 * (t + 1)
                        state[(t, h)] = dict(
                            pv=pvps.tile([128, SB], f32, tag="pv", name="pv"),
                            sum=sumps.tile([128, SB], f32, tag="sum", name="sum"),
                            e=[None] * n, n=n)
                    return state[(t, h)]

                def emit_score_exp(t, h, c):
                    st = ensure_state(t, h)
                    qs = slice(t * SB, (t + 1) * SB)
                    sc = scps.tile([128, SB], f32, tag="sc", name="sc")
                    nc.tensor.matmul(sc[:], kr[:, c * 128:(c + 1) * 128],
                                     qr[h][:, qs], start=True, stop=True)
                    j = c - 4 * t
                    if j >= 0:  # chunk contains the causal diagonal
                        nc.vector.tensor_add(sc[:], sc[:], madd_sb[:, j, :])
                    e = expp.tile([128, SB], f16, tag="e", name="e")
                    nc.scalar.activation(e[:], sc[:], AF.Exp,
                                         scale=inv_sqrt_dh, bias=ebias[:])
                    st["e"][c] = e

                def emit_mm(t, h, c):
                    st = state[(t, h)]
                    mmf = dict(start=(c == 0), stop=(c == st["n"] - 1))
                    e = st["e"][c]
                    nc.tensor.matmul(st["sum"][:], ones_h[:], e[:], **mmf)
                    nc.tensor.matmul(st["pv"][:], vnat[:, c, :], e[:], **mmf)
                    st["e"][c] = None
                    if c == st["n"] - 1:
                        emit_epilogue(t, h)

                def emit_epilogue(t, h):
                    # sum matmul used a [128,128] ones weight, so Z is already
                    # replicated across partitions: reciprocal is the broadcast.
                    st = state.pop((t, h))
                    qs = slice(t * SB, (t + 1) * SB)
                    rbc = zpool.tile([128, SB], f32, tag="rbc", name="rbc")
                    nc.vector.reciprocal(out=rbc, in_=st["sum"][:])
                    nc.vector.tensor_mul(attn[h][:, qs], st["pv"][:], rbc[:])

                work = [(t, h, c) for (t, h) in tasks
                        for c in range(4 * (t + 1))]
                prev = None
                for k, (t, h, c) in enumerate(work):
                    emit_score_exp(t, h, c)
                    if prev is not None:
                        emit_mm(*prev)
                    prev = (t, h, c)
                    if k == 3:  # after the first q-block is in flight
                        nc.sync.dma_start(
                            out=wo_r,
                            in_=woT.rearrange("(h p) d -> p h d", p=128))
                emit_mm(*prev)

            # ============= Phase 3: output projection ======================
            with tc.tile_pool(name="obuf", bufs=3) as obuf, \
                 tc.tile_pool(name="ops", bufs=3, space="PSUM") as ops:
                for st in range(N_ST):
                    rs = slice(st * 128, (st + 1) * 128)
                    for nb in range(N_NB):
                        cs = slice(nb * SB, (nb + 1) * SB)
                        op = ops.tile([128, SB], f32, tag="op")
                        for h in range(G):
                            nc.tensor.matmul(op[:], attn[h][:, rs],
                                             wo_r[:, h, cs],
                                             start=(h == 0), stop=(h == G - 1))
                        ob = obuf.tile([128, SB], f32, tag="ob")
                        nc.scalar.copy(out=ob, in_=op[:])
                        nc.sync.dma_start(out=out[rs, cs], in_=ob)

    nc.compile()
    return nc


def _prep_inputs(hidden_states, Wq, Wk, Wv, Wo, cos, sin):
    x = np.asarray(hidden_states, dtype=np.float32).reshape(S, D)
    Wq = np.asarray(Wq, dtype=np.float32)
    Wk = np.asarray(Wk, dtype=np.float32)
    Wv = np.asarray(Wv, dtype=np.float32)
    Wo = np.asarray(Wo, dtype=np.float32)
    cos = np.asarray(cos, dtype=np.float32)
    sin = np.asarray(sin, dtype=np.float32)

    xT = np.ascontiguousarray(x.T).astype(np.float16)
    cosT = np.ascontiguousarray(cos.T)
    sinmT = np.ascontiguousarray(sin.T).copy()
    sinmT[: Dh // 2] *= -1.0
    # additive causal masks for the 4 diagonal offsets, applied to raw
    # (pre-scale) scores in PSUM: -1e4 where kpos > q.
    maskadd = np.zeros((N_SB, 128, SB), dtype=np.float32)
    kp = np.arange(128)[:, None]
    qc = np.arange(SB)[None, :]
    for j in range(N_SB):
        maskadd[j] = np.where(kp + 128 * j > qc, -1e4, 0.0).astype(np.float32)

    in_maps = []
    for c in range(N_CORES):
        in_maps.append({
            "xT": xT,
            "wqT": np.ascontiguousarray(
                Wq[c * EH:(c + 1) * EH, :].T).astype(np.float16),
            "wkT": np.ascontiguousarray(
                Wk[c * Dh:(c + 1) * Dh, :].T).astype(np.float16),
            "wvT": np.ascontiguousarray(
                Wv[c * Dh:(c + 1) * Dh, :].T).astype(np.float16),
            "woT": np.ascontiguousarray(
                Wo[:, c * EH:(c + 1) * EH].T).astype(np.float16),
            "cosT": cosT,
            "sinmT": sinmT,
            "maskadd": maskadd,
        })
    return in_maps


def run(trace=False, **inputs):
    """Run on hardware; returns (full_output, exec_time_ns or None)."""
    from concourse.bass_utils import run_bass_kernel_spmd

    if trace:
        _install_ntff_hook()
    if "nc" not in _cache:
        _cache["nc"] = _build()
    nc = _cache["nc"]
    in_maps = _prep_inputs(**inputs)
    res = run_bass_kernel_spmd(nc, in_maps, core_ids=list(range(N_CORES)),
                               trace=trace)
    acc = res.results[0]["out"].astype(np.float32)
    for c in range(1, N_CORES):
        acc += res.results[c]["out"]
    return acc.reshape(B, S, D), res.exec_time_ns


def _install_ntff_hook():
    """Register the axon NTFF profiling hook missing from this image."""
    import types
    try:
        import antenv
        from trn_agent_boot.trn_boot import _ntff_profile_via_ctypes
    except ImportError:
        return
    if "antenv.axon_hooks" in sys.modules:
        return
    mod = types.ModuleType("antenv.axon_hooks")
    mod._hook = _ntff_profile_via_ctypes("/opt/axon/libaxon_pjrt.so")
    mod.get_axon_ntff_profile_hook = lambda: mod._hook
    mod.set_axon_ntff_profile_hook = lambda h: setattr(mod, "_hook", h)
    sys.modules["antenv.axon_hooks"] = mod
    antenv.axon_hooks = mod


def kernel(**inputs):
    out, _ = run(trace=False, **inputs)
    return out
